# revision 1
# baseline (speedup 1.0000x reference)
"""GAT layer (project + edge-softmax attention + aggregate + head-mean + LayerNorm + PReLU)
on 8 Trainium2 NeuronCores.

Sharding: nodes/edges partitioned by destination across the 8 cores; edges of
each core are grouped into 128-destination blocks. Per 128-edge tile the device
computes the projection h_e = x[src_e] @ W via PE, the attention logits
leaky_relu(a_src + a_dst) + exp on ACT/DVE, and the segment softmax-weighted
aggregation as a single one-hot matmul per tile accumulating into PSUM per
destination block (weights folded into the moving operand, softmax
denominators obtained from 4 extra ones-columns). Epilogue per block:
normalize, mean over heads, LayerNorm, PReLU.

The host side (input sharding) expands source features per edge slot
(x.T[:, src[slot]], fp16) so the device consumes purely sequential streams —
per-edge DMA gathers are descriptor-rate-bound (~14 ns/descriptor measured) on
TRN2 and cannot reach the memory roofline.
"""
import sys

sys.path.insert(0, "/opt/trn_rl_repo")

import numpy as np
from contextlib import ExitStack

import concourse.bass as bass
import concourse.tile as tile
from concourse import bacc, mybir
from concourse.bass_utils import run_bass_kernel_spmd

# ---- problem constants (hardcoded per harness contract) ----
N = 50000
IN_DIM = 128
OUT_DIM = 64
HEADS = 4
HC = HEADS * OUT_DIM          # 256
NEG_SLOPE = 0.2
EPS = 1e-5

NCORES = 8
ND = N // NCORES              # 6250 dst nodes per core
P = 128
NB = (ND + P - 1) // P        # 49 blocks (last has 106 dsts)
NDP = NB * P                  # 6272 padded local nodes
CH = 32                       # tiles per streamed chunk

F16 = mybir.dt.float16
F32 = mybir.dt.float32
U8 = mybir.dt.uint8

_CACHE = {}


def _build(S, T_b):
    """Compile the SPMD program. S = padded edge slots per core (mult of 128),
    T_b = tuple of per-block tile counts (len NB, sum*128 == S)."""
    n_tiles = S // P
    nchunks = (n_tiles + CH - 1) // CH

    nc = bacc.Bacc("TRN2", target_bir_lowering=False, debug=False)

    xeT = nc.dram_tensor("xeT", [P, S], F16, kind="ExternalInput")
    dstloc = nc.dram_tensor("dstloc", [P, n_tiles], F32, kind="ExternalInput")
    drep = nc.dram_tensor("drep", [S], U8, kind="ExternalInput")
    xTl = nc.dram_tensor("xTl", [P, NDP], F16, kind="ExternalInput")
    WV = nc.dram_tensor("WV", [P, HC + HEADS], F16, kind="ExternalInput")
    U16 = nc.dram_tensor("U16", [P, HEADS], F16, kind="ExternalInput")
    iota_mat = nc.dram_tensor("iota_mat", [P, P], F16, kind="ExternalInput")
    iota_col = nc.dram_tensor("iota_col", [P, 1], F32, kind="ExternalInput")
    # packed per-channel constants replicated across partitions:
    # [bias(64) | gamma(64) | beta(64) | prelu_w(1)]
    crep = nc.dram_tensor("crep", [P, 3 * OUT_DIM + 1], F32, kind="ExternalInput")
    out = nc.dram_tensor("out", [ND, OUT_DIM], F32, kind="ExternalOutput")

    RW = HC + HEADS           # 260: rhs/psum width (256 msg + 4 denom cols)

    with tile.TileContext(nc) as tc, ExitStack() as ctx:
        const_p = ctx.enter_context(tc.tile_pool(name="const", bufs=1))
        xet_p = ctx.enter_context(tc.tile_pool(name="xet", bufs=2))
        smt_p = ctx.enter_context(tc.tile_pool(name="smt", bufs=2))
        work_p = ctx.enter_context(tc.tile_pool(name="work", bufs=3))
        epi_p = ctx.enter_context(tc.tile_pool(name="epi", bufs=2))
        ph_p = ctx.enter_context(tc.tile_pool(name="ph", bufs=2, space="PSUM"))
        pm_p = ctx.enter_context(tc.tile_pool(name="pm", bufs=2, space="PSUM"))
        pa_p = ctx.enter_context(tc.tile_pool(name="pa", bufs=2, space="PSUM"))

        # ---- constants ----
        wv_s = const_p.tile([P, RW], F16)
        nc.sync.dma_start(wv_s[:], WV[:])
        u_s = const_p.tile([P, HEADS], F16)
        nc.sync.dma_start(u_s[:], U16[:])
        im_s = const_p.tile([P, P], F16)
        nc.sync.dma_start(im_s[:], iota_mat[:])
        ic_s = const_p.tile([P, 1], F32)
        nc.sync.dma_start(ic_s[:], iota_col[:])
        cr_s = const_p.tile([P, 3 * OUT_DIM + 1], F32)
        nc.sync.dma_start(cr_s[:], crep[:])
        dl_s = const_p.tile([P, n_tiles], F32)
        nc.sync.dma_start(dl_s[:], dstloc[:])
        eps_s = const_p.tile([P, 1], F32)
        nc.vector.memset(eps_s[:], EPS)
        bias_r = cr_s[:, 0:OUT_DIM]
        gamma_r = cr_s[:, OUT_DIM:2 * OUT_DIM]
        beta_r = cr_s[:, 2 * OUT_DIM:3 * OUT_DIM]
        w_prelu = cr_s[:, 3 * OUT_DIM:3 * OUT_DIM + 1]

        # ---- phase 0: a_dst for local nodes (kept in SBUF, fp16) ----
        adst_s = const_p.tile([P, NB * HEADS], F16)
        with tc.tile_pool(name="p0", bufs=1) as p0_pool, \
             tc.tile_pool(name="p0ps", bufs=2, space="PSUM") as p0_psum:
            xtl_s = p0_pool.tile([P, NDP], F16)
            nc.sync.dma_start(xtl_s[:], xTl[:])
            for b in range(NB):
                ps = p0_psum.tile([P, HEADS], F32, space="PSUM")
                nc.tensor.matmul(
                    ps[:], lhsT=xtl_s[:, b * P:(b + 1) * P], rhs=u_s[:],
                    start=True, stop=True)
                nc.scalar.copy(adst_s[:, b * HEADS:(b + 1) * HEADS], ps[:])

        # ---- main loop ----
        xet_ch = None
        smt_ch = None
        cur_c = -1

        def ensure_chunk(c):
            nonlocal xet_ch, smt_ch, cur_c
            if c == cur_c:
                return
            cur_c = c
            lo = c * CH * P
            hi = min(S, (c + 1) * CH * P)
            w = hi - lo
            xet_ch = xet_p.tile([P, CH * P], F16, tag="xet")
            nc.sync.dma_start(xet_ch[:, :w], xeT[:, lo:hi])
            dr_ch = xet_p.tile([P, CH * P], U8, tag="drep")
            nc.gpsimd.dma_start(
                dr_ch[:, :w],
                bass.AP(drep.ap().tensor, lo, [[0, P], [1, w]]))
            # S_maskT for the whole chunk: smT[d, e] = (dstloc[e] == d)
            smt_ch = smt_p.tile([P, CH * P], F16, tag="smt")
            nc.vector.tensor_scalar(
                out=smt_ch[:, :w], in0=dr_ch[:, :w],
                scalar1=ic_s[:, 0:1], scalar2=None,
                op0=mybir.AluOpType.is_equal)

        t_global = 0
        for b in range(NB):
            pm = pm_p.tile([P, RW], F32, space="PSUM")
            nt = T_b[b]
            for ti in range(nt):
                t = t_global + ti
                c, toff = divmod(t, CH)
                ensure_chunk(c)
                sl = slice(toff * P, (toff + 1) * P)

                # projection: h_e [128e, 260] = xeT_tile.T @ [W|V]
                ph = ph_p.tile([P, RW], F32, space="PSUM")
                nc.tensor.matmul(ph[:], lhsT=xet_ch[:, sl], rhs=wv_s[:],
                                 start=True, stop=True)

                # S_mask [e, d] = (iota_mat == dstloc_e)
                sm = work_p.tile([P, P], F16, tag="sm")
                nc.vector.tensor_scalar(
                    out=sm[:], in0=im_s[:], scalar1=dl_s[:, t:t + 1],
                    scalar2=None, op0=mybir.AluOpType.is_equal)

                # a_dst per edge: [e, H] = S_maskT.T @ adst_blk
                pa = pa_p.tile([P, HEADS], F32, space="PSUM")
                nc.tensor.matmul(
                    pa[:], lhsT=smt_ch[:, sl],
                    rhs=adst_s[:, b * HEADS:(b + 1) * HEADS],
                    start=True, stop=True)
                adst_e = work_p.tile([P, HEADS], F32, tag="adst_e")
                nc.scalar.copy(adst_e[:], pa[:])

                # alpha = a_src + a_dst ; lk = max(alpha, 0.2*alpha) ; e = exp(lk)
                alpha = work_p.tile([P, HEADS], F32, tag="alpha")
                nc.vector.tensor_add(alpha[:], ph[:, HC:RW], adst_e[:])
                lk = work_p.tile([P, HEADS], F32, tag="lk")
                nc.vector.scalar_tensor_tensor(
                    out=lk[:], in0=alpha[:], scalar=NEG_SLOPE, in1=alpha[:],
                    op0=mybir.AluOpType.mult, op1=mybir.AluOpType.max)
                rhs = work_p.tile([P, RW], F16, tag="rhs")
                nc.scalar.activation(rhs[:, HC:RW], lk[:],
                                     mybir.ActivationFunctionType.Exp)

                # rhs[:, :256] = h * e (per-head broadcast)
                e_b = bass.AP(rhs[:].tensor, rhs[:].offset + HC,
                              [rhs[:].ap[0], [1, HEADS], [0, OUT_DIM]])
                nc.vector.tensor_tensor(
                    out=rhs[:, 0:HC].rearrange("p (h c) -> p h c", h=HEADS),
                    in0=ph[:, 0:HC].rearrange("p (h c) -> p h c", h=HEADS),
                    in1=e_b, op=mybir.AluOpType.mult)

                # segment sum + denominators
                nc.tensor.matmul(pm[:], lhsT=sm[:], rhs=rhs[:],
                                 start=(ti == 0), stop=(ti == nt - 1))
            t_global += nt

            # ---- epilogue for block b ----
            den = epi_p.tile([P, HEADS], F32, tag="den")
            nc.vector.tensor_scalar(
                out=den[:], in0=pm[:, HC:RW], scalar1=1e-30, scalar2=None,
                op0=mybir.AluOpType.add)
            rec = epi_p.tile([P, HEADS], F32, tag="rec")
            nc.vector.reciprocal(rec[:], den[:])
            rec4 = epi_p.tile([P, HEADS], F32, tag="rec4")
            nc.vector.tensor_scalar(
                out=rec4[:], in0=rec[:], scalar1=1.0 / HEADS, scalar2=None,
                op0=mybir.AluOpType.mult)

            acc = epi_p.tile([P, OUT_DIM], F32, tag="acc")
            nc.vector.tensor_scalar(
                out=acc[:], in0=pm[:, 0:OUT_DIM], scalar1=rec4[:, 0:1],
                scalar2=None, op0=mybir.AluOpType.mult)
            for hd in range(1, HEADS):
                nc.vector.scalar_tensor_tensor(
                    out=acc[:], in0=pm[:, hd * OUT_DIM:(hd + 1) * OUT_DIM],
                    scalar=rec4[:, hd:hd + 1], in1=acc[:],
                    op0=mybir.AluOpType.mult, op1=mybir.AluOpType.add)
            nc.vector.tensor_add(acc[:], acc[:], bias_r)

            # LayerNorm over 64 channels
            stats = epi_p.tile([P, 6], F32, tag="stats")
            nc.vector.bn_stats(out=stats[:], in_=acc[:])
            mv = epi_p.tile([P, 2], F32, tag="mv")
            nc.vector.bn_aggr(out=mv[:], in_=stats[:])
            std = epi_p.tile([P, 1], F32, tag="std")
            nc.scalar.activation(std[:], mv[:, 1:2],
                                 mybir.ActivationFunctionType.Sqrt,
                                 bias=eps_s[:, 0:1])
            rstd = epi_p.tile([P, 1], F32, tag="rstd")
            nc.vector.reciprocal(rstd[:], std[:])
            xc = epi_p.tile([P, OUT_DIM], F32, tag="xc")
            nc.vector.tensor_scalar(
                out=xc[:], in0=acc[:], scalar1=mv[:, 0:1], scalar2=rstd[:, 0:1],
                op0=mybir.AluOpType.subtract, op1=mybir.AluOpType.mult)
            y = epi_p.tile([P, OUT_DIM], F32, tag="y")
            nc.vector.tensor_mul(y[:], xc[:], gamma_r)
            nc.vector.tensor_add(y[:], y[:], beta_r)

            # PReLU: max(y,0) + w*min(y,0)
            pos = epi_p.tile([P, OUT_DIM], F32, tag="pos")
            nc.vector.tensor_scalar(
                out=pos[:], in0=y[:], scalar1=0.0, scalar2=None,
                op0=mybir.AluOpType.max)
            neg = epi_p.tile([P, OUT_DIM], F32, tag="neg")
            nc.vector.tensor_scalar(
                out=neg[:], in0=y[:], scalar1=0.0, scalar2=w_prelu,
                op0=mybir.AluOpType.min, op1=mybir.AluOpType.mult)
            fin = epi_p.tile([P, OUT_DIM], F32, tag="fin")
            nc.vector.tensor_add(fin[:], pos[:], neg[:])

            rows = min(P, ND - b * P)
            nc.sync.dma_start(out[b * P:b * P + rows, :], fin[:rows, :])

    nc.compile()
    return nc


def _prep(x, edge_index, W, att_src, att_dst, bias, gamma, beta, prelu_w):
    """Host-side sharding: self-loops, dst-sort, per-core per-block padding,
    per-edge-slot source-feature expansion (fp16), weight folding."""
    src = np.concatenate([edge_index[0], np.arange(N, dtype=edge_index.dtype)])
    dst = np.concatenate([edge_index[1], np.arange(N, dtype=edge_index.dtype)])
    order = np.argsort(dst, kind="stable")
    src = src[order].astype(np.int64)
    dst = dst[order].astype(np.int64)

    # folded attention vectors: a_src = x @ V, a_dst = x @ U
    Wh = W.reshape(IN_DIM, HEADS, OUT_DIM)
    V = np.einsum("khc,hc->kh", Wh, att_src).astype(np.float64)  # [128, H]
    U = np.einsum("khc,hc->kh", Wh, att_dst)                     # [128, H]

    # pad column q: q @ V = -c for every head -> exp weight == 0
    c = 5000.0
    Q, _, _, _ = np.linalg.lstsq(V.T, -c * np.ones(HEADS), rcond=None)
    q16 = Q.astype(np.float16)
    assert np.all(np.isfinite(q16)), "pad vector overflows fp16"
    assert (q16.astype(np.float64) @ V < -500).all(), "pad logits not low enough"

    x16 = x.astype(np.float16)

    # per-core / per-block edge counts -> shared tile budget T_b
    # block of edge i: (dst // 128) within its core
    counts = np.zeros((NCORES, NB), dtype=np.int64)
    core_of = dst // ND
    blk_of = (dst % ND) // P
    np.add.at(counts, (core_of, blk_of), 1)
    T_b = tuple(int(v) for v in np.ceil(counts.max(axis=0) / P).astype(np.int64))
    S = int(sum(T_b)) * P

    in_maps = []
    iota_mat = np.tile(np.arange(P, dtype=np.float16), (P, 1))
    iota_col = np.arange(P, dtype=np.float32).reshape(P, 1)
    WV16 = np.concatenate([W, V.astype(np.float32)], axis=1).astype(np.float16)
    U16 = U.astype(np.float16)
    crep = np.zeros((P, 3 * OUT_DIM + 1), dtype=np.float32)
    crep[:, 0:OUT_DIM] = bias
    crep[:, OUT_DIM:2 * OUT_DIM] = gamma
    crep[:, 2 * OUT_DIM:3 * OUT_DIM] = beta
    crep[:, 3 * OUT_DIM] = prelu_w[0]

    slot_starts = np.concatenate([[0], np.cumsum(np.array(T_b) * P)])
    for k in range(NCORES):
        sel = core_of == k
        src_k, dst_k = src[sel], dst[sel]
        blk_k = (dst_k % ND) // P

        src_slots = np.zeros(S, dtype=np.int64)      # pad -> dummy (overwritten by q)
        pad_mask = np.ones(S, dtype=bool)
        dloc = np.full(S, 127, dtype=np.float32)     # pad dstloc
        # place each block's edges at its slot range
        o = np.argsort(blk_k, kind="stable")
        src_k, dst_k, blk_k = src_k[o], dst_k[o], blk_k[o]
        bstart = np.searchsorted(blk_k, np.arange(NB + 1))
        for b in range(NB):
            lo, hi = bstart[b], bstart[b + 1]
            n = hi - lo
            s0 = slot_starts[b]
            src_slots[s0:s0 + n] = src_k[lo:hi]
            pad_mask[s0:s0 + n] = False
            dloc[s0:s0 + n] = (dst_k[lo:hi] % ND) % P

        xe = x16[src_slots]                          # [S, 128]
        xe[pad_mask] = q16
        xeT = np.ascontiguousarray(xe.T)             # [128, S]

        dstloc_mat = np.ascontiguousarray(dloc.reshape(S // P, P).T).astype(np.float32)
        drep_u8 = dloc.astype(np.uint8)

        xTl = np.zeros((P, NDP), dtype=np.float16)
        xTl[:, :ND] = x16[k * ND:(k + 1) * ND].T

        in_maps.append({
            "xeT": xeT, "dstloc": dstloc_mat, "drep": drep_u8, "xTl": xTl,
            "WV": WV16, "U16": U16, "iota_mat": iota_mat, "iota_col": iota_col,
            "crep": crep,
        })
    return S, T_b, in_maps


def kernel(x, edge_index, W, att_src, att_dst, bias, gamma, beta, prelu_w,
           _trace=False):
    x = np.asarray(x, dtype=np.float32)
    edge_index = np.asarray(edge_index)
    S, T_b, in_maps = _prep(
        x, edge_index, np.asarray(W, np.float32), np.asarray(att_src, np.float32),
        np.asarray(att_dst, np.float32), np.asarray(bias, np.float32),
        np.asarray(gamma, np.float32), np.asarray(beta, np.float32),
        np.asarray(prelu_w, np.float32))

    key = (S, T_b)
    if key not in _CACHE:
        _CACHE[key] = _build(S, T_b)
    nc = _CACHE[key]

    res = run_bass_kernel_spmd(nc, in_maps, core_ids=list(range(NCORES)),
                               trace=_trace)
    out = np.concatenate([res.results[k]["out"] for k in range(NCORES)], axis=0)
    if _trace:
        kernel.last_exec_time_ns = res.exec_time_ns
    return out
